# revision 23
# baseline (speedup 1.0000x reference)
"""Chamfer distance kernel for Trainium2 (8 NeuronCores, SPMD).

Problem: xyz1 [4, 8192, 3], xyz2 [4, 8192, 3] (fp32 randn)
  d1[b, n] = min_m ||xyz1[b,n] - xyz2[b,m]||^2
  d2[b, m] = min_n ||xyz1[b,n] - xyz2[b,m]||^2
Returns (d1, d2), both [4, 8192] fp32.

Sharding: 8 cores = (batch b in 0..3) x (half h in 0..1).  Core (b, h)
handles queries n in [h*4096, (h+1)*4096) of batch b against the full
xyz2[b]:
  - d1 for its 4096 queries (exact),
  - a d2 partial = min over its n-half for all 8192 m (host combines
    the two halves).

Device algorithm (per core), all reductions in u-space (u = -dist/2,
so min-dist == max-u; host scales outputs by -2):
  Augmented K=24 bf16 matmul computes  psum = q.d - 0.5||d||^2
  - 0.5||q||^2 = u on the tensor engine (fp32 coords split into three
  bf16 terms; six significant cross products + split norm rows
  reconstruct the fp32 dot product to ~2^-24 at full bf16 rate).
  Engine division of labor per n-tile (4 psum groups of [128, 2048]):
    - ScalarE (the only cheap PSUM reader) evacuates every group to
      fp16 SBUF: 8192 cols/tile at 1 elem/lane/cyc (~243 us/core).
    - VectorE (the only engine with 2-tensor elementwise ops; fp16
      runs in its 2x packed mode): d1 = tree of tensor_tensor(max)
      folds, halved once more before the 1x-rate row-reduce; plus the
      d2 fold chains into persistent fp16 accumulators for the first
      `4 - npar` m-groups.
    - GpSimd (Pool): per-tile partition_all_reduce(max) for the last
      `npar` m-groups (d2), rows collected via SP-issued DMAs, plus
      the accumulators' final cross-partition max.  (TensorTensor &
      TensorReduce are ISA-illegal on Pool; tensor_tensor_reduce on
      DVE compiles but faults at runtime on this stack - hence this
      division.)
"""

import ml_dtypes
import numpy as np

import concourse.bass as bass
import concourse.mybir as mybir
import concourse.tile as tile
from concourse import bacc, bass_isa, library_config
from concourse.bass_utils import run_bass_kernel_spmd

B, N, M = 4, 8192, 8192
NCORES = 8
QH = N // 2          # queries per core (4096)
NT = QH // 128       # 32 n-tiles of 128 queries
GW = 2048            # psum group width (4 banks)
NG = M // GW         # 4 groups per n-tile

K = 24               # augmented contraction rows (bf16 triple-split)

F16 = mybir.dt.float16
F32 = mybir.dt.float32
BF16 = mybir.dt.bfloat16
MAX = mybir.AluOpType.max
AXX = mybir.AxisListType.X
NPBF = ml_dtypes.bfloat16

_cached = {}

DEFAULT_FINISH = "par"
DEFAULT_D1TTR = False
DEFAULT_NPAR = 1


def build_bass(nt=NT, reps=1, finish=None, d1ttr=None, npar=None):
    """npar: number of m-groups (from the top) whose d2 reduction runs as
    per-tile partition_all_reduce on GpSimd instead of DVE fold chains.
    finish: how the DVE-folded acc2 groups finish ('transpose' or 'par')."""
    if finish is None:
        finish = DEFAULT_FINISH
    if d1ttr is None:
        d1ttr = DEFAULT_D1TTR
    if npar is None:
        npar = DEFAULT_NPAR
    nfold = NG - npar
    assert npar == 0 or finish == "par", "npar>0 requires finish='par'"
    use_gpsimd = npar > 0 or finish == "par"
    nc = bacc.Bacc("TRN2", target_bir_lowering=False, debug=False)
    w_d = nc.dram_tensor("w", [K, QH], BF16, kind="ExternalInput").ap()
    rhs_d = nc.dram_tensor("rhs", [K, M], BF16, kind="ExternalInput").ap()
    id_d = nc.dram_tensor("ident", [128, 128], F16, kind="ExternalInput").ap()
    d1_d = nc.dram_tensor("d1", [128, NT], F32, kind="ExternalOutput").ap()
    d2_shape = [128, M // 128] if (finish == "transpose" and npar == 0) else [1, M]
    d2_d = nc.dram_tensor("d2", d2_shape, F32, kind="ExternalOutput").ap()

    with tile.TileContext(nc) as tc:
        with tc.tile_pool(name="persist", bufs=1) as pp:
            w_s = pp.tile([K, QH], BF16, tag="w_s")
            rhs_s = pp.tile([K, M], BF16, tag="rhs_s")
            id_s = pp.tile([128, 128], F16, tag="id_s")
            d1b = pp.tile([128, NT], F32, tag="d1b")
            r01 = pp.tile([128, NT], F32, tag="r01")
            if nfold:
                if finish == "par":
                    pout = pp.tile([128, nfold * GW], F32, tag="pout",
                                   name="pout")
                else:
                    d2b = pp.tile([128, nfold * GW // 128], F32, tag="d2b")
                acc2 = [
                    pp.tile([128, nfold * GW], F16, tag="acc2_0",
                            name="acc2_0"),
                    pp.tile([128, nfold * GW], F16, tag="acc2_1",
                            name="acc2_1"),
                ]
            coll = [
                pp.tile([NT, GW], F16, tag=f"coll{i}", name=f"coll{i}")
                for i in range(npar)
            ]
            cfin = [
                pp.tile([NT, GW], F32, tag=f"cfin{i}", name=f"cfin{i}")
                for i in range(npar)
            ]
            scr = pp.tile([128, GW], F16, tag="scr")  # TTR mandatory out
            nc.sync.dma_start(w_s[:], w_d)
            nc.sync.dma_start(rhs_s[:], rhs_d)
            nc.sync.dma_start(id_s[:], id_d)
            if use_gpsimd:
                # GpSimd ucode library with partition_all_reduce
                nc.gpsimd.load_library(library_config.attn)

            # Dummy 1-wait matmuls: absorb each input-DMA semaphore into
            # PE's observed clock so real matmuls never wait on DMA
            # (matmul ISA struct encodes at most one sync wait).
            with tc.tile_pool(name="dummy", bufs=1, space="PSUM") as dup:
                dm1 = dup.tile([1, 8], F32, tag="dm1", name="dm1")
                dm2 = dup.tile([1, 8], F32, tag="dm2", name="dm2")
                dm3 = dup.tile([1, 8], F32, tag="dm3", name="dm3")
                nc.tensor.matmul(dm1[0:1, 0:1], w_s[0:1, 0:1], w_s[0:1, 0:1])
                nc.tensor.matmul(dm2[0:1, 0:1], rhs_s[0:1, 0:1], rhs_s[0:1, 0:1])
                nc.tensor.matmul(dm3[0:1, 0:1], id_s[0:1, 0:1], id_s[0:1, 0:1])

            with (
                tc.tile_pool(name="psum", bufs=2, space="PSUM") as psp,
                tc.tile_pool(name="sp", bufs=8) as sp,
                tc.tile_pool(name="fp", bufs=2) as fp,
                tc.tile_pool(name="parp", bufs=2) as parp,
            ):
                for rep in range(reps):
                    s_prev = None
                    for t in range(nt):
                        lhsT = w_s[:, t * 128 : (t + 1) * 128]
                        s = []
                        for ci in range(NG):
                            pt = psp.tile([128, GW], F32, tag="pt", name="pt")
                            for cc in range(GW // 512):
                                nc.tensor.matmul(
                                    pt[:, cc * 512 : (cc + 1) * 512],
                                    lhsT,
                                    rhs_s[
                                        :,
                                        ci * GW + cc * 512 : ci * GW + (cc + 1) * 512,
                                    ],
                                    start=True,
                                    stop=True,
                                )
                            # fp16 u-tile (u = -dist/2): plain evacuation
                            sg = sp.tile([128, GW], F16, tag="s", name="sg")
                            nc.scalar.copy(sg[:], pt[:])
                            s.append(sg)
                        # d1: two fused fold+row-reduce ops; the second
                        # chains the first's row-accum as its initial value
                        if d1ttr:
                            a01 = fp.tile([128, GW], F16, tag="a01", name="a01")
                            nc.vector.tensor_tensor_reduce(
                                out=a01[:],
                                in0=s[0][:],
                                in1=s[1][:],
                                scale=1.0,
                                scalar=-3.0e4,
                                op0=MAX,
                                op1=MAX,
                                accum_out=r01[:, t : t + 1],
                            )
                            nc.vector.tensor_tensor_reduce(
                                out=scr[:],
                                in0=s[2][:],
                                in1=s[3][:],
                                scale=1.0,
                                scalar=r01[:, t : t + 1],
                                op0=MAX,
                                op1=MAX,
                                accum_out=d1b[:, t : t + 1],
                            )
                        else:
                            a01 = fp.tile([128, GW], F16, tag="a01", name="a01")
                            nc.vector.tensor_tensor(a01[:], s[0][:], s[1][:], MAX)
                            a23 = fp.tile([128, GW], F16, tag="a23", name="a23")
                            nc.vector.tensor_tensor(a23[:], s[2][:], s[3][:], MAX)
                            af = fp.tile([128, GW], F16, tag="af", name="af")
                            nc.vector.tensor_tensor(af[:], a01[:], a23[:], MAX)
                            # halve before the 1x-rate reduce
                            ah = fp.tile([128, GW // 2], F16, tag="ah",
                                         name="ah")
                            nc.vector.tensor_tensor(
                                ah[:], af[:, : GW // 2], af[:, GW // 2 :], MAX
                            )
                            nc.vector.tensor_reduce(
                                d1b[:, t : t + 1], ah[:], axis=AXX, op=MAX
                            )
                        # d2 fold groups (ping-pong) on DVE; t==1 folds
                        # t0's tiles directly (no t==0 init copies)
                        if t > 0:
                            for ci in range(nfold):
                                gs = slice(ci * GW, (ci + 1) * GW)
                                if t == 1:
                                    nc.vector.tensor_tensor(
                                        acc2[1][:, gs],
                                        s_prev[ci][:],
                                        s[ci][:],
                                        MAX,
                                    )
                                else:
                                    nc.vector.tensor_tensor(
                                        acc2[t % 2][:, gs],
                                        acc2[(t + 1) % 2][:, gs],
                                        s[ci][:],
                                        MAX,
                                    )
                        # d2 par groups: per-tile cross-partition max on
                        # GpSimd; SP-issued DMA drops row 0 into collector
                        for j in range(npar):
                            pb = parp.tile([128, GW], F16, tag="pb", name="pb")
                            nc.gpsimd.partition_all_reduce(
                                pb[:], s[nfold + j][:], channels=128,
                                reduce_op=bass_isa.ReduceOp.max,
                            )
                            nc.sync.dma_start(coll[j][t : t + 1, :], pb[0:1, :])
                        s_prev = s

                    if nfold:
                        accf = acc2[(nt - 1) % 2]
                        if finish == "par":
                            # d2 finish: one cross-partition max on GpSimd
                            nc.gpsimd.partition_all_reduce(
                                pout[:], accf[:], channels=128,
                                reduce_op=bass_isa.ReduceOp.max,
                            )
                            if npar:
                                nc.sync.dma_start(
                                    d2_d[0:1, 0 : nfold * GW], pout[0:1, :]
                                )
                        else:
                            # d2 finish: transpose acc2 in 128-col blocks
                            # (PE) and row-reduce.  Same psum pool/tag as the
                            # matmuls (a pool boundary would attach
                            # multi-wait release deps to the transposes,
                            # over the matmul ISA sync-wait budget).
                            for blk in range(nfold * GW // 128):
                                tp = psp.tile([128, 128], F16, tag="pt",
                                              name="tp")
                                nc.tensor.transpose(
                                    tp[:],
                                    accf[:, blk * 128 : (blk + 1) * 128],
                                    id_s[:],
                                )
                                nc.vector.tensor_reduce(
                                    d2b[:, blk : blk + 1], tp[:], axis=AXX,
                                    op=MAX,
                                )
                    # par groups finish: reduce the 32 collected rows
                    for j in range(npar):
                        nc.gpsimd.partition_all_reduce(
                            cfin[j][:], coll[j][:], channels=nt,
                            reduce_op=bass_isa.ReduceOp.max,
                        )
                        g0 = (nfold + j) * GW
                        nc.sync.dma_start(
                            d2_d[0:1, g0 : g0 + GW], cfin[j][0:1, :]
                        )

            nc.sync.dma_start(d1_d, d1b[:])
            if npar == 0:
                if finish == "par":
                    nc.sync.dma_start(d2_d, pout[0:1, :])
                else:
                    nc.sync.dma_start(d2_d, d2b[:])
    nc.compile()
    return nc


def _split3(x):
    """Exact 3-way bf16 split of fp32 data: x ~= s0 + s1 + s2."""
    x = np.asarray(x, np.float32)
    s0 = x.astype(NPBF)
    r1 = x - s0.astype(np.float32)
    s1 = r1.astype(NPBF)
    r2 = r1 - s1.astype(np.float32)
    s2 = r2.astype(NPBF)
    return s0, s1, s2


def _aug(pts, n_norm_sign, coord_rows, norm_rows):
    """Build the [24, npts] bf16 augmented matrix.

    coord_rows: list of 6 split-indices for the 6 coord-row triples.
    norm_rows: 'ones_then_norm' (rows 18-20 ones, 21-23 norm splits) or
               'norm_then_ones'.
    The norm value used is n_norm_sign * 0.5 * ||p||^2.
    """
    npts = pts.shape[0]
    s = _split3(pts.T)  # each [3, npts]
    out = np.zeros((K, npts), dtype=NPBF)
    for i, si in enumerate(coord_rows):
        out[3 * i : 3 * i + 3] = s[si]
    norm = (pts.astype(np.float64) ** 2).sum(-1) * 0.5
    n0, n1, n2 = _split3((n_norm_sign * norm).astype(np.float32))
    if norm_rows == "ones_then_norm":
        out[18:21] = np.asarray(1.0, NPBF)
        out[21] = n0
        out[22] = n1
        out[23] = n2
    else:
        out[18] = n0
        out[19] = n1
        out[20] = n2
        out[21:24] = np.asarray(-1.0, NPBF)
    return out


def make_inputs(xyz1, xyz2):
    """Per-core augmented input arrays.

    psum = sum_k W[k,n] * RHS[k,m]
         = (q0+q1+q2).(d0+d1+d2) [6 leading terms]
           - 0.5||d||^2 - 0.5||q||^2  =  -dist/2
    Pairings (row triples): W q0,q0,q1,q0,q2,q1 x RHS d0,d1,d0,d2,d0,d1.
    Rows 18-20: W ones x RHS -0.5||d||^2 splits.
    Rows 21-23: W +0.5||q||^2 splits x RHS -ones... (sign folded: W
    carries +0.5||q||^2 and RHS carries -1).
    """
    ident = np.eye(128, dtype=np.float16)
    in_maps = []
    for c in range(NCORES):
        b, h = divmod(c, 2)
        q = xyz1[b, h * QH : (h + 1) * QH]  # [4096, 3]
        d = xyz2[b]  # [8192, 3]
        w = _aug(q, +1.0, [0, 0, 1, 0, 2, 1], "ones_then_norm")
        # W norm rows 21-23 hold +0.5||q||^2 splits; ones rows are 18-20.
        r = _aug(d, -1.0, [0, 1, 0, 2, 0, 1], "norm_then_ones")
        in_maps.append({"w": w, "rhs": r, "ident": ident})
    return in_maps


def get_runner(nt=NT, reps=1, finish=None, d1ttr=None):
    """Build the Bass program once and wrap it in a cached jitted
    shard_map executable over the 8 cores.

    Returns (run, out_info) where run(in_maps: list[dict]) -> list of
    per-core output dicts.
    """
    ckey = ("runner", nt, reps, finish, d1ttr)
    if ckey in _cached:
        return _cached[ckey]

    import jax
    from jax.sharding import Mesh, PartitionSpec
    from jax.experimental.shard_map import shard_map
    from concourse import bass2jax, mybir as mb

    bass2jax.install_neuronx_cc_hook()
    nc = build_bass(nt=nt, reps=reps, finish=finish, d1ttr=d1ttr)

    part_name = nc.partition_id_tensor.name if nc.partition_id_tensor else None
    in_names, out_names, out_avals, zero_outs = [], [], [], []
    for alloc in nc.m.functions[0].allocations:
        if not isinstance(alloc, mb.MemoryLocationSet):
            continue
        name = alloc.memorylocations[0].name
        if alloc.kind == "ExternalInput":
            if name != part_name:
                in_names.append(name)
        elif alloc.kind == "ExternalOutput":
            out_names.append(name)
            shape = tuple(alloc.tensor_shape)
            dtype = mb.dt.np(alloc.dtype)
            out_avals.append(jax.core.ShapedArray(shape, dtype))
            zero_outs.append(np.zeros(shape, dtype))
    n_params = len(in_names)
    n_outs = len(out_names)
    all_in_names = in_names + out_names
    if part_name is not None:
        all_in_names = all_in_names + [part_name]

    def _body(*args):
        operands = list(args)
        if part_name is not None:
            operands.append(bass2jax.partition_id_tensor())
        outs = bass2jax._bass_exec_p.bind(
            *operands,
            out_avals=tuple(out_avals),
            in_names=tuple(all_in_names),
            out_names=tuple(out_names),
            lowering_input_output_aliases=(),
            sim_require_finite=True,
            sim_require_nnan=True,
            nc=nc,
        )
        return tuple(outs)

    devices = jax.devices()[:NCORES]
    mesh = Mesh(np.asarray(devices), ("core",))
    donate = tuple(range(n_params, n_params + n_outs))
    smapped = shard_map(
        _body,
        mesh=mesh,
        in_specs=(PartitionSpec("core"),) * (n_params + n_outs),
        out_specs=(PartitionSpec("core"),) * n_outs,
        check_rep=False,
    )
    sharded = jax.jit(smapped, donate_argnums=donate, keep_unused=True)

    def run(in_maps):
        per_core = [[np.asarray(m[nm]) for nm in in_names] for m in in_maps]
        concat_in = [
            np.concatenate([per_core[c][i] for c in range(NCORES)], axis=0)
            for i in range(n_params)
        ]
        concat_zeros = [
            np.zeros((NCORES * z.shape[0], *z.shape[1:]), z.dtype)
            for z in zero_outs
        ]
        out_arrs = sharded(*concat_in, *concat_zeros)
        return [
            {
                name: np.asarray(out_arrs[i]).reshape(
                    NCORES, *out_avals[i].shape
                )[c]
                for i, name in enumerate(out_names)
            }
            for c in range(NCORES)
        ]

    _cached[ckey] = (
        run,
        (in_names, out_names, out_avals, zero_outs, sharded, smapped),
    )
    return _cached[ckey]


def d2_row(arr):
    """Flatten a per-core d2 output to a [M] u-space row (either layout)."""
    arr = np.asarray(arr)
    return arr.T.reshape(M) if arr.shape[0] == 128 else arr.reshape(M)


def assemble(results):
    """Outputs are u-space (u = -dist/2) row maxes; scale by -2 here."""
    d1 = np.empty((B, N), dtype=np.float32)
    d2 = np.empty((B, M), dtype=np.float32)
    d2p = []
    for c in range(NCORES):
        b, h = divmod(c, 2)
        out = results[c]
        d1[b, h * QH : (h + 1) * QH] = -2.0 * out["d1"].T.reshape(QH)
        d2p.append(d2_row(out["d2"]))
    for b in range(B):
        d2[b] = -2.0 * np.maximum(d2p[2 * b], d2p[2 * b + 1])
    return d1, d2


def kernel(xyz1, xyz2):
    xyz1 = np.asarray(xyz1, dtype=np.float32)
    xyz2 = np.asarray(xyz2, dtype=np.float32)
    run, _ = get_runner()
    results = run(make_inputs(xyz1, xyz2))
    return assemble(results)


# revision 29
# speedup vs baseline: 1.3170x; 1.3170x over previous
"""Chamfer distance kernel for Trainium2 (8 NeuronCores, SPMD).

Problem: xyz1 [4, 8192, 3], xyz2 [4, 8192, 3] (fp32 randn)
  d1[b, n] = min_m ||xyz1[b,n] - xyz2[b,m]||^2
  d2[b, m] = min_n ||xyz1[b,n] - xyz2[b,m]||^2
Returns (d1, d2), both [4, 8192] fp32.

Sharding: 8 cores = (batch b in 0..3) x (half h in 0..1).  Core (b, h)
handles queries n in [h*4096, (h+1)*4096) of batch b against the full
xyz2[b]:
  - d1 for its 4096 queries (exact),
  - a d2 partial = min over its n-half for all 8192 m (host combines
    the two halves).

Device algorithm (per core), all reductions in u-space (u = -dist/2,
so min-dist == max-u; host scales outputs by -2):
  Augmented K=24 bf16 matmul computes  psum = q.d - 0.5||d||^2
  - 0.5||q||^2 = u on the tensor engine (fp32 coords split into three
  bf16 terms; six significant cross products + split norm rows
  reconstruct the fp32 dot product to ~2^-24 at full bf16 rate).
  Engine division of labor per n-tile (4 psum groups of [128, 2048]):
    - ScalarE (the only cheap PSUM reader) evacuates every group to
      fp16 SBUF: 8192 cols/tile at 1 elem/lane/cyc (~243 us/core).
    - VectorE (the only engine with 2-tensor elementwise ops; fp16
      runs in its 2x packed mode): d1 = tree of tensor_tensor(max)
      folds, halved once more before the 1x-rate row-reduce; plus the
      d2 fold chains into persistent fp16 accumulators for the first
      `4 - npar` m-groups.
    - GpSimd (Pool): per-tile partition_all_reduce(max) for the last
      `npar` m-groups (d2), rows collected via SP-issued DMAs, plus
      the accumulators' final cross-partition max.  (TensorTensor &
      TensorReduce are ISA-illegal on Pool; tensor_tensor_reduce on
      DVE compiles but faults at runtime on this stack - hence this
      division.)
"""

import ml_dtypes
import numpy as np

import concourse.bass as bass
import concourse.mybir as mybir
import concourse.tile as tile
from concourse import bacc, bass_isa, library_config
from concourse.bass_utils import run_bass_kernel_spmd

B, N, M = 4, 8192, 8192
NCORES = 8
QH = N // 2          # queries per core (4096)
NT = QH // 128       # 32 n-tiles of 128 queries
GW = 2048            # psum group width (4 banks)
NG = M // GW         # 4 groups per n-tile

K = 24               # augmented contraction rows (bf16 triple-split)

F16 = mybir.dt.float16
F32 = mybir.dt.float32
BF16 = mybir.dt.bfloat16
MAX = mybir.AluOpType.max
AXX = mybir.AxisListType.X
NPBF = ml_dtypes.bfloat16

_cached = {}

DEFAULT_FINISH = "par"
DEFAULT_D1TTR = False
DEFAULT_NPAR = 1


def build_bass(nt=NT, reps=1, finish=None, d1ttr=None, npar=None):
    """npar: number of m-groups (from the top) whose d2 reduction runs as
    per-tile partition_all_reduce on GpSimd instead of DVE fold chains.
    finish: how the DVE-folded acc2 groups finish ('transpose' or 'par')."""
    if finish is None:
        finish = DEFAULT_FINISH
    if d1ttr is None:
        d1ttr = DEFAULT_D1TTR
    if npar is None:
        npar = DEFAULT_NPAR
    nfold = NG - npar
    assert npar == 0 or finish == "par", "npar>0 requires finish='par'"
    use_gpsimd = npar > 0 or finish == "par"
    nc = bacc.Bacc("TRN2", target_bir_lowering=False, debug=False)
    w_d = nc.dram_tensor("w", [K, QH], BF16, kind="ExternalInput").ap()
    rhs_d = nc.dram_tensor("rhs", [K, M], BF16, kind="ExternalInput").ap()
    id_d = nc.dram_tensor("ident", [128, 128], F16, kind="ExternalInput").ap()
    d1_d = nc.dram_tensor("d1", [128, NT], F32, kind="ExternalOutput").ap()
    d2_shape = [128, M // 128] if (finish == "transpose" and npar == 0) else [1, M]
    d2_d = nc.dram_tensor("d2", d2_shape, F32, kind="ExternalOutput").ap()

    with tile.TileContext(nc) as tc:
        with tc.tile_pool(name="persist", bufs=1) as pp:
            w_s = pp.tile([K, QH], BF16, tag="w_s")
            rhs_s = pp.tile([K, M], BF16, tag="rhs_s")
            id_s = pp.tile([128, 128], F16, tag="id_s")
            d1b = pp.tile([128, NT], F32, tag="d1b")
            if nfold:
                if finish == "par":
                    pout = pp.tile([128, nfold * GW], F32, tag="pout",
                                   name="pout")
                else:
                    d2b = pp.tile([128, nfold * GW // 128], F32, tag="d2b")
                acc2 = [
                    pp.tile([128, nfold * GW], F16, tag="acc2_0",
                            name="acc2_0"),
                    pp.tile([128, nfold * GW], F16, tag="acc2_1",
                            name="acc2_1"),
                ]
            coll = [
                pp.tile([NT, GW], F16, tag=f"coll{i}", name=f"coll{i}")
                for i in range(npar)
            ]
            cfin = [
                pp.tile([NT, GW], F32, tag=f"cfin{i}", name=f"cfin{i}")
                for i in range(npar)
            ]
            nc.sync.dma_start(w_s[:], w_d)
            nc.sync.dma_start(rhs_s[:], rhs_d)
            nc.sync.dma_start(id_s[:], id_d)
            if use_gpsimd:
                # GpSimd ucode library with partition_all_reduce
                nc.gpsimd.load_library(library_config.attn)

            # Dummy 1-wait matmuls: absorb each input-DMA semaphore into
            # PE's observed clock so real matmuls never wait on DMA
            # (matmul ISA struct encodes at most one sync wait).
            with tc.tile_pool(name="dummy", bufs=1, space="PSUM") as dup:
                dm1 = dup.tile([1, 8], F32, tag="dm1", name="dm1")
                dm2 = dup.tile([1, 8], F32, tag="dm2", name="dm2")
                dm3 = dup.tile([1, 8], F32, tag="dm3", name="dm3")
                nc.tensor.matmul(dm1[0:1, 0:1], w_s[0:1, 0:1], w_s[0:1, 0:1])
                nc.tensor.matmul(dm2[0:1, 0:1], rhs_s[0:1, 0:1], rhs_s[0:1, 0:1])
                nc.tensor.matmul(dm3[0:1, 0:1], id_s[0:1, 0:1], id_s[0:1, 0:1])

            with (
                tc.tile_pool(name="psum", bufs=2, space="PSUM") as psp,
                tc.tile_pool(name="sp", bufs=4) as sp,
                tc.tile_pool(name="fp", bufs=2) as fp,
                tc.tile_pool(name="parp", bufs=2) as parp,
            ):
                for rep in range(reps):
                    s_prev = None
                    for t in range(nt):
                        lhsT = w_s[:, t * 128 : (t + 1) * 128]
                        # evac pairs of psum groups into [128, 2*GW] fp16
                        # tiles (u = -dist/2); s01 = groups 0|1, s23 = 2|3
                        s = []
                        for half in range(2):
                            sg = sp.tile(
                                [128, 2 * GW], F16,
                                tag=f"s{half}", name=f"s{half}",
                            )
                            for sub in range(2):
                                ci = 2 * half + sub
                                pt = psp.tile([128, GW], F32, tag="pt",
                                              name="pt")
                                for cc in range(GW // 512):
                                    nc.tensor.matmul(
                                        pt[:, cc * 512 : (cc + 1) * 512],
                                        lhsT,
                                        rhs_s[
                                            :,
                                            ci * GW + cc * 512 :
                                            ci * GW + (cc + 1) * 512,
                                        ],
                                        start=True,
                                        stop=True,
                                    )
                                nc.scalar.copy(
                                    sg[:, sub * GW : (sub + 1) * GW], pt[:]
                                )
                            s.append(sg)
                        # d1: one [128, 4096] 4-group fold, then a halving
                        # tree (fp16 folds run 2x; the final reduce is 1x,
                        # so shrink its input first)
                        a = fp.tile([128, 2 * GW], F16, tag="a", name="a")
                        nc.vector.tensor_tensor(a[:], s[0][:], s[1][:], MAX)
                        hw_, src = 2 * GW, a
                        while hw_ > 512:
                            hw_ //= 2
                            h = fp.tile([128, hw_], F16, tag=f"h{hw_}",
                                        name=f"h{hw_}")
                            nc.vector.tensor_tensor(
                                h[:], src[:, :hw_], src[:, hw_ : 2 * hw_], MAX
                            )
                            src = h
                        nc.vector.tensor_reduce(
                            d1b[:, t : t + 1], src[:], axis=AXX, op=MAX
                        )

                        # d2 fold-group slices of the pair tiles, widest APs
                        # possible: [(tile, col-slice), ...] covering nfold*GW
                        def fold_slices(sl):
                            if nfold == 0:
                                return []
                            if nfold == 1:
                                return [(sl[0][:, 0:GW], 0)]
                            out = [(sl[0][:, :], 0)]
                            if nfold == 3:
                                out.append((sl[1][:, 0:GW], 2 * GW))
                            elif nfold == 4:
                                out.append((sl[1][:, :], 2 * GW))
                            return out

                        # d2 fold groups (ping-pong) on DVE; t==1 folds
                        # t0's tiles directly (no t==0 init copies)
                        if t > 0:
                            for (ap, base), (papp, _) in zip(
                                fold_slices(s), fold_slices(s_prev)
                            ):
                                w = ap.shape[1]
                                gs = slice(base, base + w)
                                if t == 1:
                                    nc.vector.tensor_tensor(
                                        acc2[1][:, gs], papp, ap, MAX
                                    )
                                else:
                                    nc.vector.tensor_tensor(
                                        acc2[t % 2][:, gs],
                                        acc2[(t + 1) % 2][:, gs],
                                        ap,
                                        MAX,
                                    )
                        # d2 par groups: per-tile cross-partition max on
                        # GpSimd; SP-issued DMA drops row 0 into collector
                        for j in range(npar):
                            ci = nfold + j
                            pin = s[ci // 2][:, (ci % 2) * GW : (ci % 2 + 1) * GW]
                            pb = parp.tile([128, GW], F16, tag="pb", name="pb")
                            nc.gpsimd.partition_all_reduce(
                                pb[:], pin, channels=128,
                                reduce_op=bass_isa.ReduceOp.max,
                            )
                            nc.sync.dma_start(coll[j][t : t + 1, :], pb[0:1, :])
                        s_prev = s

                    if nfold:
                        accf = acc2[(nt - 1) % 2]
                        if finish == "par":
                            # d2 finish: one cross-partition max on GpSimd
                            nc.gpsimd.partition_all_reduce(
                                pout[:], accf[:], channels=128,
                                reduce_op=bass_isa.ReduceOp.max,
                            )
                            if npar:
                                nc.sync.dma_start(
                                    d2_d[0:1, 0 : nfold * GW], pout[0:1, :]
                                )
                        else:
                            # d2 finish: transpose acc2 in 128-col blocks
                            # (PE) and row-reduce.  Same psum pool/tag as the
                            # matmuls (a pool boundary would attach
                            # multi-wait release deps to the transposes,
                            # over the matmul ISA sync-wait budget).
                            for blk in range(nfold * GW // 128):
                                tp = psp.tile([128, 128], F16, tag="pt",
                                              name="tp")
                                nc.tensor.transpose(
                                    tp[:],
                                    accf[:, blk * 128 : (blk + 1) * 128],
                                    id_s[:],
                                )
                                nc.vector.tensor_reduce(
                                    d2b[:, blk : blk + 1], tp[:], axis=AXX,
                                    op=MAX,
                                )
                    # par groups finish: reduce the 32 collected rows
                    for j in range(npar):
                        nc.gpsimd.partition_all_reduce(
                            cfin[j][:], coll[j][:], channels=nt,
                            reduce_op=bass_isa.ReduceOp.max,
                        )
                        g0 = (nfold + j) * GW
                        nc.sync.dma_start(
                            d2_d[0:1, g0 : g0 + GW], cfin[j][0:1, :]
                        )

            nc.sync.dma_start(d1_d, d1b[:])
            if npar == 0:
                if finish == "par":
                    nc.sync.dma_start(d2_d, pout[0:1, :])
                else:
                    nc.sync.dma_start(d2_d, d2b[:])
    nc.compile()
    return nc


def _split3(x):
    """Exact 3-way bf16 split of fp32 data: x ~= s0 + s1 + s2."""
    x = np.asarray(x, np.float32)
    s0 = x.astype(NPBF)
    r1 = x - s0.astype(np.float32)
    s1 = r1.astype(NPBF)
    r2 = r1 - s1.astype(np.float32)
    s2 = r2.astype(NPBF)
    return s0, s1, s2


def _aug(pts, n_norm_sign, coord_rows, norm_rows):
    """Build the [24, npts] bf16 augmented matrix.

    coord_rows: list of 6 split-indices for the 6 coord-row triples.
    norm_rows: 'ones_then_norm' (rows 18-20 ones, 21-23 norm splits) or
               'norm_then_ones'.
    The norm value used is n_norm_sign * 0.5 * ||p||^2.
    """
    npts = pts.shape[0]
    s = _split3(pts.T)  # each [3, npts]
    out = np.zeros((K, npts), dtype=NPBF)
    for i, si in enumerate(coord_rows):
        out[3 * i : 3 * i + 3] = s[si]
    norm = (pts.astype(np.float64) ** 2).sum(-1) * 0.5
    n0, n1, n2 = _split3((n_norm_sign * norm).astype(np.float32))
    if norm_rows == "ones_then_norm":
        out[18:21] = np.asarray(1.0, NPBF)
        out[21] = n0
        out[22] = n1
        out[23] = n2
    else:
        out[18] = n0
        out[19] = n1
        out[20] = n2
        out[21:24] = np.asarray(-1.0, NPBF)
    return out


def make_inputs(xyz1, xyz2):
    """Per-core augmented input arrays.

    psum = sum_k W[k,n] * RHS[k,m]
         = (q0+q1+q2).(d0+d1+d2) [6 leading terms]
           - 0.5||d||^2 - 0.5||q||^2  =  -dist/2
    Pairings (row triples): W q0,q0,q1,q0,q2,q1 x RHS d0,d1,d0,d2,d0,d1.
    Rows 18-20: W ones x RHS -0.5||d||^2 splits.
    Rows 21-23: W +0.5||q||^2 splits x RHS -ones... (sign folded: W
    carries +0.5||q||^2 and RHS carries -1).
    """
    ident = np.eye(128, dtype=np.float16)
    in_maps = []
    for c in range(NCORES):
        b, h = divmod(c, 2)
        q = xyz1[b, h * QH : (h + 1) * QH]  # [4096, 3]
        d = xyz2[b]  # [8192, 3]
        w = _aug(q, +1.0, [0, 0, 1, 0, 2, 1], "ones_then_norm")
        # W norm rows 21-23 hold +0.5||q||^2 splits; ones rows are 18-20.
        r = _aug(d, -1.0, [0, 1, 0, 2, 0, 1], "norm_then_ones")
        in_maps.append({"w": w, "rhs": r, "ident": ident})
    return in_maps


def get_runner(nt=NT, reps=1, finish=None, d1ttr=None):
    """Build the Bass program once and wrap it in a cached jitted
    shard_map executable over the 8 cores.

    Returns (run, out_info) where run(in_maps: list[dict]) -> list of
    per-core output dicts.
    """
    ckey = ("runner", nt, reps, finish, d1ttr)
    if ckey in _cached:
        return _cached[ckey]

    import jax
    from jax.sharding import Mesh, PartitionSpec
    from jax.experimental.shard_map import shard_map
    from concourse import bass2jax, mybir as mb

    bass2jax.install_neuronx_cc_hook()
    nc = build_bass(nt=nt, reps=reps, finish=finish, d1ttr=d1ttr)

    part_name = nc.partition_id_tensor.name if nc.partition_id_tensor else None
    in_names, out_names, out_avals, zero_outs = [], [], [], []
    for alloc in nc.m.functions[0].allocations:
        if not isinstance(alloc, mb.MemoryLocationSet):
            continue
        name = alloc.memorylocations[0].name
        if alloc.kind == "ExternalInput":
            if name != part_name:
                in_names.append(name)
        elif alloc.kind == "ExternalOutput":
            out_names.append(name)
            shape = tuple(alloc.tensor_shape)
            dtype = mb.dt.np(alloc.dtype)
            out_avals.append(jax.core.ShapedArray(shape, dtype))
            zero_outs.append(np.zeros(shape, dtype))
    n_params = len(in_names)
    n_outs = len(out_names)
    all_in_names = in_names + out_names
    if part_name is not None:
        all_in_names = all_in_names + [part_name]

    def _body(*args):
        operands = list(args)
        if part_name is not None:
            operands.append(bass2jax.partition_id_tensor())
        outs = bass2jax._bass_exec_p.bind(
            *operands,
            out_avals=tuple(out_avals),
            in_names=tuple(all_in_names),
            out_names=tuple(out_names),
            lowering_input_output_aliases=(),
            sim_require_finite=True,
            sim_require_nnan=True,
            nc=nc,
        )
        return tuple(outs)

    devices = jax.devices()[:NCORES]
    mesh = Mesh(np.asarray(devices), ("core",))
    donate = tuple(range(n_params, n_params + n_outs))
    smapped = shard_map(
        _body,
        mesh=mesh,
        in_specs=(PartitionSpec("core"),) * (n_params + n_outs),
        out_specs=(PartitionSpec("core"),) * n_outs,
        check_rep=False,
    )
    sharded = jax.jit(smapped, donate_argnums=donate, keep_unused=True)

    def run(in_maps):
        per_core = [[np.asarray(m[nm]) for nm in in_names] for m in in_maps]
        concat_in = [
            np.concatenate([per_core[c][i] for c in range(NCORES)], axis=0)
            for i in range(n_params)
        ]
        concat_zeros = [
            np.zeros((NCORES * z.shape[0], *z.shape[1:]), z.dtype)
            for z in zero_outs
        ]
        out_arrs = sharded(*concat_in, *concat_zeros)
        return [
            {
                name: np.asarray(out_arrs[i]).reshape(
                    NCORES, *out_avals[i].shape
                )[c]
                for i, name in enumerate(out_names)
            }
            for c in range(NCORES)
        ]

    _cached[ckey] = (
        run,
        (in_names, out_names, out_avals, zero_outs, sharded, smapped),
    )
    return _cached[ckey]


def d2_row(arr):
    """Flatten a per-core d2 output to a [M] u-space row (either layout)."""
    arr = np.asarray(arr)
    return arr.T.reshape(M) if arr.shape[0] == 128 else arr.reshape(M)


def assemble(results):
    """Outputs are u-space (u = -dist/2) row maxes; scale by -2 here."""
    d1 = np.empty((B, N), dtype=np.float32)
    d2 = np.empty((B, M), dtype=np.float32)
    d2p = []
    for c in range(NCORES):
        b, h = divmod(c, 2)
        out = results[c]
        d1[b, h * QH : (h + 1) * QH] = -2.0 * out["d1"].T.reshape(QH)
        d2p.append(d2_row(out["d2"]))
    for b in range(B):
        d2[b] = -2.0 * np.maximum(d2p[2 * b], d2p[2 * b + 1])
    return d1, d2


def kernel(xyz1, xyz2):
    xyz1 = np.asarray(xyz1, dtype=np.float32)
    xyz2 = np.asarray(xyz2, dtype=np.float32)
    run, _ = get_runner()
    results = run(make_inputs(xyz1, xyz2))
    return assemble(results)
